# revision 1
# baseline (speedup 1.0000x reference)
"""Trainium2 Bass kernel for MultiHeadSelfAttention (B=8, C=512, H=W=32, 8 heads).

Sharding: data-parallel — one batch element per NeuronCore (8 cores).

Per-core math (batch element b, S = H*W = 1024 tokens, C = 512 channels):
  x_flat = x[b].reshape(C, S)          # [C, S], token s = column
  LN over C; gamma/beta folded into effective QKV weights on host.
  q/k computed transposed [o, s]; v computed [t, d]; scores computed
  transposed [t, s] so the softmax denominator rides the PV matmul as an
  augmented ones-column; query-side score-bias terms cancel in softmax and
  the key-side term is applied as the per-partition bias of the exp
  activation. All matmuls run as float32r (full-rate on TRN2 for moving
  dim >= 256), accumulation in fp32 PSUM.

Host-side weight prep (numpy, not part of HW time):
  W'x = Wx * gamma  (x in {q,k,v});  b'x = bx + Wx @ beta
  wqk = [W'q.T | W'k.T]  [C, 1024];  w1 = row-sums of W' (mu fold)
"""

import math

import numpy as np

C = 512
S = 1024
B = 8
NH = 8
HD = 64
N_CORES = 8

_CACHE = {}


def _build_nc(repeat=1, debug=False):
    import concourse.bass as bass
    import concourse.mybir as mybir
    import concourse.tile as tile
    from concourse import bacc

    f32 = mybir.dt.float32
    f32r = mybir.dt.float32r
    AF = mybir.ActivationFunctionType
    OP = mybir.AluOpType

    nc = bacc.Bacc("TRN2", debug=False, num_devices=N_CORES)

    x_d = nc.declare_dram_parameter("x", [C, S], f32r, isOutput=False)
    wqk_d = nc.declare_dram_parameter("wqk", [C, 2 * C], f32r, isOutput=False)
    wv_d = nc.declare_dram_parameter("wv", [C, C], f32r, isOutput=False)
    wo_d = nc.declare_dram_parameter("wo", [C, C], f32r, isOutput=False)
    nw1qk_d = nc.declare_dram_parameter("nw1qk", [1, 2 * C], f32r, isOutput=False)
    nw1v_d = nc.declare_dram_parameter("nw1v", [1, C], f32r, isOutput=False)
    ucol_d = nc.declare_dram_parameter("ucol", [128, 4], f32r, isOutput=False)
    nc0_d = nc.declare_dram_parameter("nc0", [1, 1], f32, isOutput=False)
    bocol_d = nc.declare_dram_parameter("bocol", [128, 4], f32, isOutput=False)
    emat_d = nc.declare_dram_parameter("emat", [2, 128], f32r, isOutput=False)
    y_d = nc.declare_dram_parameter("y", [C, S], f32, isOutput=True)
    scr_r = nc.dram_tensor("scr_r", [1, S], f32)
    scr_g = nc.dram_tensor("scr_g", [1, S], f32)
    dbg = {}
    if debug:
        dbg["mu"] = nc.declare_dram_parameter("dbg_mu", [1, S], f32, isOutput=True)
        dbg["r"] = nc.declare_dram_parameter("dbg_r", [1, S], f32, isOutput=True)
        dbg["qt"] = nc.declare_dram_parameter("dbg_qt", [128, 4 * S], f32, isOutput=True)
        dbg["kt"] = nc.declare_dram_parameter("dbg_kt", [128, 4 * S], f32, isOutput=True)
        dbg["v"] = nc.declare_dram_parameter("dbg_v", [128, 8 * 8 * 72], f32, isOutput=True)
        dbg["sig"] = nc.declare_dram_parameter("dbg_sig", [8, S], f32, isOutput=True)
        dbg["gcol"] = nc.declare_dram_parameter("dbg_gcol", [128, 8], f32, isOutput=True)
        dbg["opk"] = nc.declare_dram_parameter("dbg_opk", [128, 4 * S], f32, isOutput=True)

    def r32(ap):
        return ap.bitcast(f32r)

    with tile.TileContext(nc) as tc:
        import contextlib

        with contextlib.ExitStack() as ctx:
            ctx.enter_context(nc.allow_low_precision(reason="f32r-rounded matmul inputs"))
            const = ctx.enter_context(tc.tile_pool(name="const", bufs=1))
            big = ctx.enter_context(tc.tile_pool(name="big", bufs=1))
            xsq_pool = ctx.enter_context(tc.tile_pool(name="xsq", bufs=2 if repeat == 1 else 1))
            pt_pool = ctx.enter_context(tc.tile_pool(name="pt", bufs=3 if repeat == 1 else 2))
            ost_pool = ctx.enter_context(tc.tile_pool(name="ost", bufs=3 if repeat == 1 else 2))
            y_pool = ctx.enter_context(tc.tile_pool(name="ysb", bufs=2))
            stats_sb = ctx.enter_context(tc.tile_pool(name="stats_sb", bufs=1))
            sg_pool = ctx.enter_context(tc.tile_pool(name="sg", bufs=2))
            ps = ctx.enter_context(tc.tile_pool(name="ps", bufs=2, space="PSUM"))

            # ---- static loads (x first, chunked, so stats start early) --------
            xpool = ctx.enter_context(tc.tile_pool(name="xpool", bufs=1 if repeat == 1 else 2))
            xsb = xpool.tile([128, 4, S], f32r, tag="xping")
            x_re = x_d[:, :].rearrange("(kc p) s -> p kc s", p=128)
            for kc in range(4):
                nc.sync.dma_start(out=xsb[:, kc, :], in_=x_re[:, kc, :])
            wqk_sb = const.tile([128, 4, 2 * C], f32r)
            wqk_re = wqk_d[:, :].rearrange("(kc p) o -> p kc o", p=128)
            nc.sync.dma_start(out=wqk_sb[:, :, 512:], in_=wqk_re[:, :, 512:])
            nc.sync.dma_start(out=wqk_sb[:, :, 0:512], in_=wqk_re[:, :, 0:512])
            wv_sb = const.tile([128, 4, C], f32r)
            nc.sync.dma_start(out=wv_sb[:], in_=wv_d[:, :].rearrange("(kc p) o -> p kc o", p=128))
            wo_sb = const.tile([128, 4, C], f32r)
            nc.sync.dma_start(out=wo_sb[:], in_=wo_d[:, :].rearrange("(kc p) o -> p kc o", p=128))
            nw1qk_sb = const.tile([1, 2 * C], f32r)
            nc.sync.dma_start(out=nw1qk_sb[:], in_=nw1qk_d[:, :])
            nw1v_sb = const.tile([1, C], f32r)
            nc.sync.dma_start(out=nw1v_sb[:], in_=nw1v_d[:, :])
            ucol_sb = const.tile([128, 4], f32r)
            nc.sync.dma_start(out=ucol_sb[:], in_=ucol_d[:, :])
            nc0_sb = const.tile([1, 1], f32)
            nc.sync.dma_start(out=nc0_sb[:], in_=nc0_d[:, :])
            bocol_sb = const.tile([128, 4], f32)
            nc.sync.dma_start(out=bocol_sb[:], in_=bocol_d[:, :])
            emat_sb = const.tile([66, 128], f32r)
            nc.sync.dma_start(out=emat_sb[64:66, :], in_=emat_d[:, :])

            ones1f = const.tile([1, 128], f32)
            nc.vector.memset(ones1f[:], 1.0)
            ones1 = const.tile([1, 128], f32r)
            nc.vector.tensor_copy(ones1[:], ones1f[:])
            onescf = const.tile([128, 1], f32)
            nc.vector.memset(onescf[:], 1.0)
            onesc = const.tile([128, 1], f32r)
            nc.vector.tensor_copy(onesc[:], onescf[:])
            # sigma-augmentation pattern: head h gets a 1 at col 64+(h%2)
            augf = const.tile([128, 8, 8], f32)
            nc.vector.memset(augf[:], 0.0)
            _ab = augf[:, 0, 0:1]
            _d0 = bass.AP(tensor=_ab.tensor, offset=_ab.offset,
                          ap=[list(_ab.ap[0]), [16, 4]])
            nc.vector.memset(_d0, 1.0)
            _ab1 = augf[:, 1, 1:2]
            _d1 = bass.AP(tensor=_ab1.tensor, offset=_ab1.offset,
                          ap=[list(_ab1.ap[0]), [16, 4]])
            nc.vector.memset(_d1, 1.0)
            augr = const.tile([128, 8, 8], f32r)
            nc.vector.tensor_copy(augr[:], augf[:])

            def psA(name):
                return ps.tile([128, S], f32, tag="psA", name=name)

            def psB(name):
                return ps.tile([128, 512], f32, tag="psB", name=name)

            def psC(name):
                return ps.tile([72, 512], f32, tag="psC", name=name)

            def body(src_sb, dst_dram, it):
                """One attention layer: src_sb [128, 4, S] -> dst_dram [C, S]."""
                # ---- stats: per-token sums of x and x^2 via PE ones-matmuls.
                # Sum(x) accumulates at psum row 0, Sum(x^2) at row 32 of the
                # same bank (separate col-groups via tile_position).
                murow = stats_sb.tile([1, S], f32r, tag="murow")
                srowA = stats_sb.tile([1, S], f32, tag="srowA")
                sts = [psB(f"stx{it}_{sc}") for sc in range(2)]
                for kc in range(4):
                    for sc in range(2):
                        nc.tensor.matmul(
                            sts[sc][0:1, :],
                            r32(onesc[:]),
                            r32(src_sb[:, kc, sc * 512:(sc + 1) * 512]),
                            start=(kc == 0), stop=(kc == 3),
                        )
                for sc in range(2):
                    nc.vector.tensor_scalar_mul(
                        murow[:, sc * 512:(sc + 1) * 512], sts[sc][0:1, :], 1.0 / C)
                stq = [psB(f"stq{it}_{sc}") for sc in range(2)]
                for kc in range(4):
                    for sc in range(2):
                        xsq = xsq_pool.tile([128, 512], f32r)
                        nc.vector.tensor_mul(
                            xsq[:], src_sb[:, kc, sc * 512:(sc + 1) * 512].bitcast(f32),
                            src_sb[:, kc, sc * 512:(sc + 1) * 512].bitcast(f32))
                        nc.tensor.matmul(
                            stq[sc][0:1, :],
                            r32(onesc[:]),
                            r32(xsq[:]),
                            start=(kc == 0), stop=(kc == 3),
                        )
                for sc in range(2):
                    nc.vector.tensor_scalar_mul(
                        srowA[:, sc * 512:(sc + 1) * 512], stq[sc][0:1, :], 1.0 / C)
                srowB = stats_sb.tile([1, S], f32, tag="srowB")
                nc.vector.tensor_mul(srowB[:], murow[:].bitcast(f32), murow[:].bitcast(f32))
                nc.vector.tensor_tensor(srowA[:], srowA[:], srowB[:], OP.subtract)
                epsr = stats_sb.tile([1, 1], f32, tag="epsr")
                nc.vector.memset(epsr[:], 1e-5)
                nc.scalar.activation(srowB[:], srowA[:], AF.Ln, bias=epsr[:], scale=1.0)
                rrow = stats_sb.tile([1, S], f32r, tag="rrow")
                nc.scalar.activation(rrow[:], srowB[:], AF.Exp, bias=0.0, scale=-0.5)

                # ---- gamma_t = u . x_t with u = W'k^T b'q (host);
                #      gamma = r * (u.x - c0*mu) * 0.125,  c0 = b'q.w1k ------
                gpsA = psA(f"g{it}")
                for sc in range(2):
                    for kc in range(4):
                        nc.tensor.matmul(
                            gpsA[0:1, sc * 512:(sc + 1) * 512],
                            r32(ucol_sb[:, kc:kc + 1]),
                            r32(src_sb[:, kc, sc * 512:(sc + 1) * 512]),
                            start=(kc == 0), stop=(kc == 3),
                        )
                g1row = stats_sb.tile([1, S], f32, tag="g1row")
                nc.vector.scalar_tensor_tensor(
                    g1row[:], murow[:].bitcast(f32), nc0_sb[:], gpsA[0:1, :],
                    OP.mult, OP.add)
                grow = stats_sb.tile([1, S], f32, tag="grow")
                nc.vector.scalar_tensor_tensor(
                    grow[:], rrow[:].bitcast(f32), 0.125, g1row[:],
                    OP.mult, OP.mult)
                gcol = stats_sb.tile([128, 8], f32, tag="gcol")
                nc.sync.dma_start(out=scr_g[:, :], in_=grow[:])
                nc.sync.dma_start(out=gcol[:], in_=scr_g[:, :].rearrange("o (t p) -> (o p) t", p=128))

                rcol = stats_sb.tile([128, 8], f32, tag="rcol")
                nc.sync.dma_start(out=scr_r[:, :], in_=rrow[:].bitcast(f32))
                nc.sync.dma_start(out=rcol[:], in_=scr_r[:, :].rearrange("o (t p) -> (o p) t", p=128))

                # R = rrow broadcast to 128 partitions (via K=1 matmul)
                ps_r = psA(f"ps_r{it}")
                for sc in range(2):
                    nc.tensor.matmul(
                        ps_r[:, sc * 512:(sc + 1) * 512],
                        r32(ones1[:]),
                        r32(rrow[:, sc * 512:(sc + 1) * 512]),
                        start=True, stop=True,
                    )
                R_sb = big.tile([128, S], f32, tag="R")
                nc.vector.tensor_copy(R_sb[:], ps_r[:])

                # ---- QK projections, transposed layout [o, s] ----------------
                # K chunks first: gamma (score bias) needs all of kt, and the
                # attention pipeline can then start while Q/V still project.
                qt_sb = big.tile([128, 4, S], f32r, tag="qt")
                kt_sb = big.tile([128, 4, S], f32r, tag="kt")
                def qk_chunk(oc):
                    dst = qt_sb if oc < 4 else kt_sb
                    o4 = oc % 4
                    for sc in range(2):
                        p = psB(f"qk{it}_{oc}_{sc}")
                        for kc in range(4):
                            nc.tensor.matmul(
                                p[:],
                                r32(wqk_sb[:, kc, oc * 128:(oc + 1) * 128]),
                                r32(src_sb[:, kc, sc * 512:(sc + 1) * 512]),
                                start=(kc == 0), stop=False,
                            )
                        nc.tensor.matmul(
                            p[:],
                            r32(nw1qk_sb[:, oc * 128:(oc + 1) * 128]),
                            r32(murow[:, sc * 512:(sc + 1) * 512]),
                            start=False, stop=True,
                        )
                        nc.vector.tensor_tensor(
                            dst[:, o4, sc * 512:(sc + 1) * 512], p[:],
                            R_sb[:, sc * 512:(sc + 1) * 512], OP.mult,
                        )

                for oc in [4, 5, 6, 7]:
                    qk_chunk(oc)


                v_sb = big.tile([128, 8, 8, 72], f32r, tag="v")

                def v_chunk(tcn):
                    p = psB(f"v{it}_{tcn}")
                    for kc in range(4):
                        nc.tensor.matmul(
                            p[:],
                            r32(src_sb[:, kc, tcn * 128:(tcn + 1) * 128]),
                            r32(wv_sb[:, kc, :]),
                            start=(kc == 0), stop=False,
                        )
                    nc.tensor.matmul(
                        p[:],
                        r32(murow[:, tcn * 128:(tcn + 1) * 128]),
                        r32(nw1v_sb[:]),
                        start=False, stop=True,
                    )
                    # v = psum * r_t  (b_v folded into the host-side bo)
                    nc.vector.tensor_scalar_mul(
                        v_sb[:, tcn, :, 0:64],
                        p[:].rearrange("p (h d) -> p h d", h=8),
                        rcol[:, tcn:tcn + 1],
                    )
                    nc.vector.tensor_copy(v_sb[:, tcn, :, 64:72], augr[:].bitcast(f32))

                if debug and it == 0:
                    nc.sync.dma_start(out=dbg["mu"][:, :], in_=murow[:].bitcast(f32))
                    nc.sync.dma_start(out=dbg["r"][:, :], in_=rrow[:].bitcast(f32))
                    nc.sync.dma_start(out=dbg["kt"][:, :], in_=kt_sb[:].bitcast(f32).rearrange("p a s -> p (a s)"))
                    nc.sync.dma_start(out=dbg["gcol"][:, :], in_=gcol[:])

                # ---- attention: per head-pair / s-chunk, transposed scores ----
                # V chunks are emitted inside pair 0; Q chunk pr just before
                # pair pr; pair pr-1's normalization is emitted mid pair pr so
                # its sigma-chain latency hides under the exp stream.
                opk_sb = big.tile([128, 4, S], f32r, tag="opk")
                sig_st = {}

                def emit_norm(pr):
                    sig_stage = sig_st.pop(pr)
                    siginv = sg_pool.tile([66, S], f32r, tag="siginv", name=f"siginv{it}_{pr}")
                    nc.vector.reciprocal(siginv[64:66, :], sig_stage[64:66, :])
                    ps_e = psA(f"pe{it}_{pr}")
                    for sc in range(2):
                        nc.tensor.matmul(
                            ps_e[:, sc * 512:(sc + 1) * 512],
                            r32(emat_sb[64:66, :]),
                            siginv[64:66, sc * 512:(sc + 1) * 512],
                            start=True, stop=True,
                        )
                    nc.vector.tensor_tensor(opk_sb[:, pr, :], ps_e[:], opk_sb[:, pr, :].bitcast(f32), OP.mult)
                    if debug and it == 0:
                        nc.sync.dma_start(out=dbg["sig"][2 * pr:2 * pr + 2, :], in_=sig_stage[64:66, :])

                for pr in range(4):
                    qk_chunk(pr)
                    sig_stage = sg_pool.tile([66, S], f32, tag="sigst", name=f"sigst{it}_{pr}")
                    for sc in range(2):
                        pvp = [psC(f"pv{it}_{pr}_{sc}_{hi}") for hi in range(2)]
                        for tcn in range(8):
                            if pr == 0 and sc == 0:
                                v_chunk(tcn)
                            pst = psA(f"sc{it}_{pr}_{sc}_{tcn}")
                            for hi in range(2):
                                b0 = 64 * hi
                                nc.tensor.matmul(
                                    pst[:, hi * 512:(hi + 1) * 512],
                                    r32(kt_sb[b0:b0 + 64, pr, tcn * 128:(tcn + 1) * 128]),
                                    r32(qt_sb[b0:b0 + 64, pr, sc * 512:(sc + 1) * 512]),
                                    start=True, stop=True,
                                )
                            pt = pt_pool.tile([128, S], f32r)
                            nc.scalar.activation(
                                pt[:], pst[:], AF.Exp,
                                bias=gcol[:, tcn:tcn + 1], scale=0.125,
                            )
                            for hi in range(2):
                                h = 2 * pr + hi
                                nc.tensor.matmul(
                                    pvp[hi][0:66, :],
                                    r32(v_sb[:, tcn, h, 0:66]),
                                    r32(pt[:, hi * 512:(hi + 1) * 512]),
                                    start=(tcn == 0), stop=(tcn == 7),
                                )
                            if pr > 0 and sc == 0 and tcn == 4:
                                emit_norm(pr - 1)
                        for hi in range(2):
                            ost = ost_pool.tile([64, 512], f32)
                            nc.vector.tensor_copy(ost[:], pvp[hi][0:64, :])
                            if hi == 0:
                                # rows 64/65 hold (sigma_h0, 0)
                                nc.vector.tensor_copy(
                                    sig_stage[64:66, sc * 512:(sc + 1) * 512],
                                    pvp[hi][64:66, :],
                                )
                            else:
                                # rows 64/65 hold (0, sigma_h1): accumulate
                                nc.vector.tensor_tensor(
                                    sig_stage[64:66, sc * 512:(sc + 1) * 512],
                                    sig_stage[64:66, sc * 512:(sc + 1) * 512],
                                    pvp[hi][64:66, :], OP.add,
                                )
                            nc.sync.dma_start(
                                out=opk_sb[64 * hi:64 * hi + 64, pr, sc * 512:(sc + 1) * 512].bitcast(f32),
                                in_=ost[:],
                            )
                    sig_st[pr] = sig_stage
                emit_norm(3)
                if debug and it == 0:
                    nc.sync.dma_start(out=dbg["qt"][:, :], in_=qt_sb[:].bitcast(f32).rearrange("p a s -> p (a s)"))
                    nc.sync.dma_start(out=dbg["v"][:, :], in_=v_sb[:].bitcast(f32).rearrange("p a h d -> p (a h d)"))
                    nc.sync.dma_start(out=dbg["opk"][:, :], in_=opk_sb[:].bitcast(f32).rearrange("p a s -> p (a s)"))
                # ---- output projection + bias + residual ----------------------
                out_tiles = []
                for c in range(4):
                    for sc in range(2):
                        if (c * 2 + sc) % 2 == 0:
                            ps_y = psB(f"y{it}_{c}_{sc}")
                        else:
                            ps_y = psA(f"y{it}_{c}_{sc}")[:, 0:512]
                        for oc in range(4):
                            nc.tensor.matmul(
                                ps_y[:],
                                r32(wo_sb[:, oc, c * 128:(c + 1) * 128]),
                                r32(opk_sb[:, oc, sc * 512:(sc + 1) * 512]),
                                start=(oc == 0), stop=(oc == 3),
                            )
                        ysb = y_pool.tile([128, 512], f32)
                        nc.vector.scalar_tensor_tensor(
                            ysb[:], ps_y[:], bocol_sb[:, c:c + 1],
                            src_sb[:, c, sc * 512:(sc + 1) * 512].bitcast(f32),
                            OP.add, OP.add,
                        )
                        nc.sync.dma_start(
                            out=dst_dram[c * 128:(c + 1) * 128, sc * 512:(sc + 1) * 512],
                            in_=ysb[:],
                        )
                        out_tiles.append(ysb)
                return out_tiles

            if repeat == 1:
                body(xsb, y_d, 0)
            else:
                cur = xsb
                for it in range(repeat):
                    outs = body(cur, y_d, it)
                    if it < repeat - 1:
                        nxt = xpool.tile([128, 4, S], f32r, tag="xping", name=f"xping{it}")
                        # reload this iteration's output as the next input
                        # (true DRAM dependency chains the iterations)
                        k = 0
                        for c in range(4):
                            for sc in range(2):
                                nc.vector.tensor_copy(
                                    nxt[:, c, sc * 512:(sc + 1) * 512], outs[k][:]
                                )
                                k += 1
                        cur = nxt

    nc.finalize()
    return nc


def _host_prep(Wq, bq, Wk, bk, Wv, bv, Wo, bo, gamma, beta):
    g = np.asarray(gamma, np.float64)
    be = np.asarray(beta, np.float64)

    def eff(W, b):
        W = np.asarray(W, np.float64)
        b = np.asarray(b, np.float64)
        Wp = W * g[None, :]
        bp = b + W @ be
        return Wp, bp

    Wqp, bqp = eff(Wq, bq)
    Wkp, bkp = eff(Wk, bk)
    Wvp, bvp = eff(Wv, bv)
    f32 = np.float32

    wqk = np.concatenate([Wqp.T, Wkp.T], axis=1).astype(f32)
    w1qk = np.concatenate([Wqp.sum(1), Wkp.sum(1)])
    nw1qk = (-w1qk)[None, :].astype(f32)
    wv = Wvp.T.astype(f32)
    nw1v = (-Wvp.sum(1))[None, :].astype(f32)
    u = Wkp.T @ bqp
    ucol = u.reshape(4, 128).T.astype(f32).copy()
    nc0 = np.array([[-(bqp @ Wkp.sum(1))]], np.float32)
    wo = np.asarray(Wo, f32).T.copy()
    bo_eff = np.asarray(bo, np.float64) + np.asarray(Wo, np.float64) @ bvp
    bocol = bo_eff.reshape(4, 128).T.astype(f32).copy()
    emat = np.zeros((2, 128), f32)
    emat[0, :64] = 1.0
    emat[1, 64:] = 1.0
    return dict(wqk=wqk, wv=wv, wo=wo, nw1qk=nw1qk, nw1v=nw1v,
                ucol=ucol, nc0=nc0, bocol=bocol, emat=emat)


def get_nc(repeat=1, debug=False):
    key = (repeat, debug)
    if key not in _CACHE:
        _CACHE[key] = _build_nc(repeat, debug)
    return _CACHE[key]


def make_in_maps(inputs):
    shared = _host_prep(
        inputs["Wq"], inputs["bq"], inputs["Wk"], inputs["bk"],
        inputs["Wv"], inputs["bv"], inputs["Wo"], inputs["bo"],
        inputs["gamma"], inputs["beta"],
    )
    x = np.asarray(inputs["x"], np.float32)
    in_maps = []
    for b in range(N_CORES):
        m = dict(shared)
        m["x"] = np.ascontiguousarray(x[b].reshape(C, S))
        in_maps.append(m)
    return in_maps


def kernel(**inputs):
    from concourse.bass_utils import run_bass_kernel_spmd

    nc = get_nc(repeat=1)
    in_maps = make_in_maps(inputs)
    res = run_bass_kernel_spmd(nc, in_maps, list(range(N_CORES)))
    out = np.stack([res.results[b]["y"].reshape(C, 32, 32) for b in range(N_CORES)])
    return out.astype(np.float32)



# revision 10
# speedup vs baseline: 1.1466x; 1.1466x over previous
"""Trainium2 Bass kernel for MultiHeadSelfAttention (B=8, C=512, H=W=32, 8 heads).

Sharding: data-parallel - one batch element per NeuronCore (8 cores).

All matmuls run in bf16 (measured ~131ns per 512-free matmul on this HW vs
~1850ns for f32r), fp32 PSUM accumulation. Per-core math for batch b
(S = 1024 tokens as columns, C = 512 channels split over 4 chunks of 128
partitions):

  xb   = bf16(x)                      # stats + residual
  mu_s = sum_c xb / C   (PE ones-matmul), var = E[x^2] - mu^2
  r_s  = exp(-0.5 ln(var+eps))        # ACT Ln+Exp (one act table)
  xc   = bf16(xb - mu)                # centered input, DVE 2x bf16
  qt/kt[o, s] = bf16(r_s * (Wqk^T xc))   # transposed projections
  v[t, hd]    = bf16(r_t * (xc^T Wv))    # parity-split layout (see below)
  scores[t, s] = kt^T-block @ qt-block  (per head, K=64)
  P = bf16(exp(0.125*scores + gamma_t))  # gamma: key-side bias approx
  O^T = v-aug^T @ P  accumulated over t-chunks; augmented ones column
        yields sigma rows; even heads' output lands at psum rows 0-63,
        odd heads' at 64-127 (sigma at rows 64 / 63) so PSUM->SBUF
        evacuation needs no partition-shift DMA (ACT Copy activations).
  O normalized by 1/sigma (broadcast via 2-row matmul), out-proj + bias
  + residual in fp32.

Host-side prep: gamma/beta folded into effective weights; Wv columns
permuted to (parity, head//2, d) so the even/odd v writes are plain APs.
"""

import math

import numpy as np

C = 512
S = 1024
B = 8
NH = 8
HD = 64
N_CORES = 8

_CACHE = {}


def _build_nc(repeat=1):
    import concourse.bass as bass
    import concourse.mybir as mybir
    import concourse.tile as tile
    from concourse import bacc

    f32 = mybir.dt.float32
    bf16 = mybir.dt.bfloat16
    AF = mybir.ActivationFunctionType
    OP = mybir.AluOpType

    nc = bacc.Bacc("TRN2", debug=False, num_devices=N_CORES)

    x_d = nc.declare_dram_parameter("x", [C, S], f32, isOutput=False)
    wqk_d = nc.declare_dram_parameter("wqk", [C, 2 * C], bf16, isOutput=False)
    wv_d = nc.declare_dram_parameter("wv", [C, C], bf16, isOutput=False)
    wo_d = nc.declare_dram_parameter("wo", [C, C], bf16, isOutput=False)
    ucol_d = nc.declare_dram_parameter("ucol", [128, 4], bf16, isOutput=False)
    bocol_d = nc.declare_dram_parameter("bocol", [128, 4], f32, isOutput=False)
    emat_d = nc.declare_dram_parameter("emat", [2, 128], bf16, isOutput=False)
    y_d = nc.declare_dram_parameter("y", [C, S], f32, isOutput=True)
    scr_g = nc.dram_tensor("scr_g", [1, S], f32)
    scr_r = nc.dram_tensor("scr_r", [1, S], f32)

    with tile.TileContext(nc) as tc:
        import contextlib

        with contextlib.ExitStack() as ctx:
            ctx.enter_context(nc.allow_low_precision(reason="bf16 matmul pipeline"))
            const = ctx.enter_context(tc.tile_pool(name="const", bufs=1))
            vpool = ctx.enter_context(tc.tile_pool(name="vpool", bufs=1))
            big = ctx.enter_context(
                tc.tile_pool(name="big", bufs=1 if repeat == 1 else 2)
            )
            xb_pool = ctx.enter_context(
                tc.tile_pool(name="xb", bufs=1 if repeat == 1 else 2)
            )
            xc_pool = ctx.enter_context(
                tc.tile_pool(name="xc", bufs=1 if repeat == 1 else 2)
            )
            xsq_pool = ctx.enter_context(tc.tile_pool(name="xsq", bufs=2))
            pt_pool = ctx.enter_context(
                tc.tile_pool(name="pt", bufs=3 if repeat == 1 else 2)
            )
            y_pool = ctx.enter_context(tc.tile_pool(name="ysb", bufs=2))
            stats_sb = ctx.enter_context(tc.tile_pool(name="stats_sb", bufs=1))
            sg_pool = ctx.enter_context(tc.tile_pool(name="sg", bufs=2))
            ps = ctx.enter_context(tc.tile_pool(name="ps", bufs=2, space="PSUM"))

            # ---- static loads ------------------------------------------------
            xfp = const.tile([128, 4, S], f32, tag="xf")
            x_re = x_d[:, :].rearrange("(kc p) s -> p kc s", p=128)
            for kc in range(4):
                nc.sync.dma_start(out=xfp[:, kc, :], in_=x_re[:, kc, :])
            wqk_sb = const.tile([128, 4, 2 * C], bf16)
            wqk_re = wqk_d[:, :].rearrange("(kc p) o -> p kc o", p=128)
            nc.sync.dma_start(out=wqk_sb[:, :, 512:], in_=wqk_re[:, :, 512:])
            nc.sync.dma_start(out=wqk_sb[:, :, 0:512], in_=wqk_re[:, :, 0:512])
            wv_sb = const.tile([128, 4, C], bf16)
            nc.sync.dma_start(
                out=wv_sb[:], in_=wv_d[:, :].rearrange("(kc p) o -> p kc o", p=128)
            )
            wo_sb = const.tile([128, 4, C], bf16)
            nc.sync.dma_start(
                out=wo_sb[:], in_=wo_d[:, :].rearrange("(kc p) o -> p kc o", p=128)
            )
            ucol_sb = const.tile([128, 4], bf16)
            nc.sync.dma_start(out=ucol_sb[:], in_=ucol_d[:, :])
            bocol_sb = const.tile([128, 4], f32)
            nc.sync.dma_start(out=bocol_sb[:], in_=bocol_d[:, :])
            emat_sb = const.tile([66, 128], bf16)
            nc.sync.dma_start(out=emat_sb[64:66, :], in_=emat_d[:, :])

            ones1f = const.tile([1, 128], f32)
            nc.vector.memset(ones1f[:], 1.0)
            ones1 = const.tile([1, 128], bf16)
            nc.vector.tensor_copy(ones1[:], ones1f[:])
            onescf = const.tile([128, 1], f32)
            nc.vector.memset(onescf[:], 1.0)
            onesc = const.tile([128, 1], bf16)
            nc.vector.tensor_copy(onesc[:], onescf[:])
            epsr = const.tile([1, 1], f32)
            nc.vector.memset(epsr[:], 1e-5)

            # v layout: [128 t, tcn, parity, hh, 66]; head h = 2*hh + parity.
            # cols 0-63 data; col 64+parity holds sigma-ones (the other is 0)
            # so the PV psum rows 64:66 accumulate [sigma_even; sigma_odd].
            v_sb = vpool.tile([128, 8, 2, 4, 66], bf16)
            nc.vector.memset(v_sb[:, :, :, :, 64:66], 0.0)
            nc.vector.memset(v_sb[:, :, 0, :, 64:65], 1.0)
            nc.vector.memset(v_sb[:, :, 1, :, 65:66], 1.0)

            # initial x -> bf16 on gpsimd
            xb0 = xb_pool.tile([128, 4, S], bf16, tag="xb")
            for kc in range(4):
                for sc in range(2):
                    nc.gpsimd.tensor_copy(
                        xb0[:, kc, sc * 512 : (sc + 1) * 512],
                        xfp[:, kc, sc * 512 : (sc + 1) * 512],
                    )

            def psA(name):
                return ps.tile([128, S], f32, tag="psA", name=name)

            def psB(name):
                return ps.tile([128, 512], f32, tag="psB", name=name)

            def psC(name):
                return ps.tile([128, 512], f32, tag="psC", name=name)

            def blk(sc):
                return slice(sc * 512, (sc + 1) * 512)

            def body(xb, dst_dram, it):
                """One attention layer: xb [128, 4, S] bf16 -> dst_dram [C, S]."""
                # ---- stats: mu ------------------------------------------------
                sts = [psB(f"stx{it}_{sc}") for sc in range(2)]
                for kc in range(4):
                    for sc in range(2):
                        nc.tensor.matmul(
                            sts[sc][0:1, :],
                            onesc[:],
                            xb[:, kc, blk(sc)],
                            start=(kc == 0),
                            stop=(kc == 3),
                        )
                murow_f = stats_sb.tile([1, S], f32, tag="murow_f")
                murow_b = stats_sb.tile([1, S], bf16, tag="murow_b")
                for sc in range(2):
                    nc.vector.tensor_scalar_mul(
                        murow_f[:, blk(sc)], sts[sc][0:1, :], 1.0 / C
                    )
                    nc.vector.tensor_scalar_mul(
                        murow_b[:, blk(sc)], sts[sc][0:1, :], 1.0 / C
                    )
                # Mu broadcast to 128 partitions, then center x
                Mu_bf = big.tile([128, S], bf16, tag="Mu")
                for sc in range(2):
                    pm = psB(f"mu{it}_{sc}")
                    nc.tensor.matmul(
                        pm[:], ones1[:], murow_b[:, blk(sc)], start=True, stop=True
                    )
                    nc.vector.tensor_copy(Mu_bf[:, blk(sc)], pm[:])
                xc = xc_pool.tile([128, 4, S], bf16, tag="xc")
                for kc in range(4):
                    for sc in range(2):
                        nc.vector.tensor_tensor(
                            xc[:, kc, blk(sc)],
                            xb[:, kc, blk(sc)],
                            Mu_bf[:, blk(sc)],
                            OP.subtract,
                        )
                # ---- var + r --------------------------------------------------
                stq = [psB(f"stq{it}_{sc}") for sc in range(2)]
                for kc in range(4):
                    for sc in range(2):
                        xsq = xsq_pool.tile([128, 512], bf16)
                        nc.vector.tensor_mul(
                            xsq[:], xb[:, kc, blk(sc)], xb[:, kc, blk(sc)]
                        )
                        nc.tensor.matmul(
                            stq[sc][0:1, :],
                            onesc[:],
                            xsq[:],
                            start=(kc == 0),
                            stop=(kc == 3),
                        )
                srowA = stats_sb.tile([1, S], f32, tag="srowA")
                for sc in range(2):
                    nc.vector.tensor_scalar_mul(
                        srowA[:, blk(sc)], stq[sc][0:1, :], 1.0 / C
                    )
                srowB = stats_sb.tile([1, S], f32, tag="srowB")
                nc.vector.tensor_mul(srowB[:], murow_f[:], murow_f[:])
                nc.vector.tensor_tensor(srowA[:], srowA[:], srowB[:], OP.subtract)
                nc.scalar.activation(srowB[:], srowA[:], AF.Ln, bias=epsr[:], scale=1.0)
                rrow = stats_sb.tile([1, S], bf16, tag="rrow")
                nc.scalar.activation(rrow[:], srowB[:], AF.Exp, bias=0.0, scale=-0.5)
                # R broadcast + rcol transpose
                R_sb = big.tile([128, S], f32, tag="R")
                for sc in range(2):
                    pr_ = psB(f"rb{it}_{sc}")
                    nc.tensor.matmul(
                        pr_[:], ones1[:], rrow[:, blk(sc)], start=True, stop=True
                    )
                    nc.vector.tensor_copy(R_sb[:, blk(sc)], pr_[:])
                rrow_f = stats_sb.tile([1, S], f32, tag="rrow_f")
                nc.vector.tensor_copy(rrow_f[:], rrow[:])
                rcol = stats_sb.tile([128, 8], f32, tag="rcol")
                nc.sync.dma_start(out=scr_r[:, :], in_=rrow_f[:])
                nc.sync.dma_start(
                    out=rcol[:], in_=scr_r[:, :].rearrange("o (t p) -> (o p) t", p=128)
                )
                # ---- gamma: key-side softmax bias -----------------------------
                gps = psA(f"g{it}")
                for sc in range(2):
                    for kc in range(4):
                        nc.tensor.matmul(
                            gps[0:1, blk(sc)],
                            ucol_sb[:, kc : kc + 1],
                            xc[:, kc, blk(sc)],
                            start=(kc == 0),
                            stop=(kc == 3),
                        )
                grow = stats_sb.tile([1, S], f32, tag="grow")
                nc.vector.scalar_tensor_tensor(
                    grow[:], rrow[:], 0.125, gps[0:1, :], OP.mult, OP.mult
                )
                gcol = stats_sb.tile([128, 8], f32, tag="gcol")
                nc.sync.dma_start(out=scr_g[:, :], in_=grow[:])
                nc.sync.dma_start(
                    out=gcol[:], in_=scr_g[:, :].rearrange("o (t p) -> (o p) t", p=128)
                )

                # ---- Q/K projections, transposed layout [o, s] ----------------
                qt_sb = big.tile([128, 4, S], bf16, tag="qt")
                kt_sb = big.tile([128, 4, S], bf16, tag="kt")

                def qk_chunk(oc):
                    dst = qt_sb if oc < 4 else kt_sb
                    o4 = oc % 4
                    for sc in range(2):
                        p = psB(f"qk{it}_{oc}_{sc}")
                        for kc in range(4):
                            nc.tensor.matmul(
                                p[:],
                                wqk_sb[:, kc, oc * 128 : (oc + 1) * 128],
                                xc[:, kc, blk(sc)],
                                start=(kc == 0),
                                stop=(kc == 3),
                            )
                        nc.vector.tensor_tensor(
                            dst[:, o4, blk(sc)], p[:], R_sb[:, blk(sc)], OP.mult
                        )

                for oc in [4, 5, 6, 7]:
                    qk_chunk(oc)

                def v_chunk(tcn):
                    p = psB(f"v{it}_{tcn}")
                    for kc in range(4):
                        nc.tensor.matmul(
                            p[:],
                            xc[:, kc, tcn * 128 : (tcn + 1) * 128],
                            wv_sb[:, kc, :],
                            start=(kc == 0),
                            stop=(kc == 3),
                        )
                    pr_ = p[:].rearrange("p (i h d) -> p i h d", i=2, h=4)
                    nc.vector.tensor_scalar_mul(
                        v_sb[:, tcn, :, :, 0:64], pr_[:, :, :, :], rcol[:, tcn : tcn + 1]
                    )

                # ---- attention ------------------------------------------------
                opk_sb = big.tile([128, 4, S], bf16, tag="opk")
                sig_st = {}

                def emit_norm(pr):
                    sig_stage = sig_st.pop(pr)
                    siginv = sg_pool.tile(
                        [66, S], bf16, tag="siginv", name=f"siginv{it}_{pr}"
                    )
                    nc.vector.reciprocal(siginv[64:66, :], sig_stage[64:66, :])
                    ps_e = psA(f"pe{it}_{pr}")
                    for sc in range(2):
                        nc.tensor.matmul(
                            ps_e[:, blk(sc)],
                            emat_sb[64:66, :],
                            siginv[64:66, blk(sc)],
                            start=True,
                            stop=True,
                        )
                    nc.vector.tensor_tensor(
                        opk_sb[:, pr, :], ps_e[:], opk_sb[:, pr, :], OP.mult
                    )

                for pr in range(4):
                    qk_chunk(pr)
                    sig_stage = sg_pool.tile(
                        [66, S], f32, tag="sigst", name=f"sigst{it}_{pr}"
                    )
                    for sc in range(2):
                        pva = psC(f"pva{it}_{pr}_{sc}")
                        pvb = psC(f"pvb{it}_{pr}_{sc}")
                        for tcn in range(8):
                            if pr == 0 and sc == 0:
                                v_chunk(tcn)
                            pst = psA(f"sc{it}_{pr}_{sc}_{tcn}")
                            for hi in range(2):
                                b0 = 64 * hi
                                nc.tensor.matmul(
                                    pst[:, hi * 512 : (hi + 1) * 512],
                                    kt_sb[b0 : b0 + 64, pr, tcn * 128 : (tcn + 1) * 128],
                                    qt_sb[b0 : b0 + 64, pr, blk(sc)],
                                    start=True,
                                    stop=True,
                                )
                            pt = pt_pool.tile([128, S], bf16)
                            nc.scalar.activation(
                                pt[:],
                                pst[:],
                                AF.Exp,
                                bias=gcol[:, tcn : tcn + 1],
                                scale=0.125,
                            )
                            nc.tensor.matmul(
                                pva[0:66, :],
                                v_sb[:, tcn, 0, pr, 0:66],
                                pt[:, 0:512],
                                start=(tcn == 0),
                                stop=(tcn == 7),
                            )
                            nc.tensor.matmul(
                                pvb[0:66, :],
                                v_sb[:, tcn, 1, pr, 0:66],
                                pt[:, 512:1024],
                                start=(tcn == 0),
                                stop=(tcn == 7),
                            )
                            if pr > 0 and sc == 0 and tcn == 4:
                                emit_norm(pr - 1)
                        # evacuate PV psum: even-head data lands directly at
                        # opk rows 0-63 (ACT copy); odd-head data needs the
                        # 64-partition shift, done via SBUF->SBUF DMA.
                        nc.scalar.activation(
                            opk_sb[0:64, pr, blk(sc)], pva[0:64, :], AF.Copy,
                            bias=0.0, scale=1.0,
                        )
                        ost = y_pool.tile(
                            [64, 512], bf16, tag="ost", name=f"ost{it}_{pr}_{sc}"
                        )
                        nc.scalar.activation(
                            ost[:], pvb[0:64, :], AF.Copy, bias=0.0, scale=1.0
                        )
                        nc.sync.dma_start(
                            out=opk_sb[64:128, pr, blk(sc)], in_=ost[:]
                        )
                        nc.vector.tensor_copy(
                            sig_stage[64:66, blk(sc)], pva[64:66, :]
                        )
                        nc.vector.tensor_tensor(
                            sig_stage[64:66, blk(sc)],
                            sig_stage[64:66, blk(sc)],
                            pvb[64:66, :],
                            OP.add,
                        )
                    sig_st[pr] = sig_stage
                emit_norm(3)

                # ---- output projection + bias + residual ----------------------
                nxt = None
                if it is not None and it < repeat - 1:
                    nxt = xb_pool.tile([128, 4, S], bf16, tag="xb", name=f"xb{it + 1}")
                for c in range(4):
                    for sc in range(2):
                        if (c * 2 + sc) % 2 == 0:
                            ps_y = psB(f"y{it}_{c}_{sc}")
                        else:
                            ps_y = psA(f"y{it}_{c}_{sc}")[:, 0:512]
                        for oc in range(4):
                            nc.tensor.matmul(
                                ps_y[:],
                                wo_sb[:, oc, c * 128 : (c + 1) * 128],
                                opk_sb[:, oc, blk(sc)],
                                start=(oc == 0),
                                stop=(oc == 3),
                            )
                        ysb = y_pool.tile([128, 512], f32)
                        nc.vector.scalar_tensor_tensor(
                            ysb[:],
                            ps_y[:],
                            bocol_sb[:, c : c + 1],
                            xb[:, c, blk(sc)],
                            OP.add,
                            OP.add,
                        )
                        nc.sync.dma_start(
                            out=dst_dram[c * 128 : (c + 1) * 128, blk(sc)],
                            in_=ysb[:],
                        )
                        if nxt is not None:
                            nc.gpsimd.tensor_copy(nxt[:, c, blk(sc)], ysb[:])
                return nxt

            cur = xb0
            for it in range(repeat):
                cur = body(cur, y_d, it)

    nc.finalize()
    return nc


def _host_prep(Wq, bq, Wk, bk, Wv, bv, Wo, bo, gamma, beta):
    import ml_dtypes

    bfd = ml_dtypes.bfloat16
    g = np.asarray(gamma, np.float64)
    be = np.asarray(beta, np.float64)

    def eff(W, b):
        W = np.asarray(W, np.float64)
        b = np.asarray(b, np.float64)
        return W * g[None, :], b + W @ be

    Wqp, bqp = eff(Wq, bq)
    Wkp, bkp = eff(Wk, bk)
    Wvp, bvp = eff(Wv, bv)

    wqk = np.concatenate([Wqp.T, Wkp.T], axis=1).astype(bfd)
    # permute Wv columns to (parity, head//2, d) order
    wvt = Wvp.T.reshape(C, 8, 64)
    perm = [2 * hh + par for par in range(2) for hh in range(4)]
    wv = np.ascontiguousarray(wvt[:, perm, :].reshape(C, C)).astype(bfd)
    u = Wkp.T @ bqp
    ucol = u.reshape(4, 128).T.astype(bfd).copy()
    wo = np.ascontiguousarray(np.asarray(Wo, np.float64).T).astype(bfd)
    bo_eff = np.asarray(bo, np.float64) + np.asarray(Wo, np.float64) @ bvp
    bocol = bo_eff.reshape(4, 128).T.astype(np.float32).copy()
    # emat rows land at stationary partitions 64 (even sigma) / 65 (odd sigma)
    emat = np.zeros((2, 128), bfd)
    emat[0, :64] = 1.0  # partition 64 = 1/sigma_even -> even head rows 0-63
    emat[1, 64:] = 1.0  # partition 65 = 1/sigma_odd -> odd head rows 64-127
    return dict(wqk=wqk, wv=wv, wo=wo, ucol=ucol, bocol=bocol, emat=emat)


def get_nc(repeat=1):
    if repeat not in _CACHE:
        _CACHE[repeat] = _build_nc(repeat)
    return _CACHE[repeat]


def make_in_maps(inputs):
    shared = _host_prep(
        inputs["Wq"], inputs["bq"], inputs["Wk"], inputs["bk"],
        inputs["Wv"], inputs["bv"], inputs["Wo"], inputs["bo"],
        inputs["gamma"], inputs["beta"],
    )
    x = np.asarray(inputs["x"], np.float32)
    in_maps = []
    for b in range(N_CORES):
        m = dict(shared)
        m["x"] = np.ascontiguousarray(x[b].reshape(C, S))
        in_maps.append(m)
    return in_maps


def kernel(**inputs):
    from concourse.bass_utils import run_bass_kernel_spmd

    nc = get_nc(repeat=1)
    in_maps = make_in_maps(inputs)
    res = run_bass_kernel_spmd(nc, in_maps, list(range(N_CORES)))
    out = np.stack([res.results[b]["y"].reshape(C, 32, 32) for b in range(N_CORES)])
    return out.astype(np.float32)


# revision 23
# speedup vs baseline: 3.1752x; 2.7692x over previous
"""Trainium2 Bass kernel for MultiHeadSelfAttention (B=8, C=512, H=W=32, 8 heads).

Sharding: data-parallel - one batch element per NeuronCore (8 cores).

All matmuls run in bf16 (measured ~131ns per 512-free matmul on this HW vs
~1850ns for f32r), fp32 PSUM accumulation. Per-core math for batch b
(S = 1024 tokens as columns, C = 512 channels split over 4 chunks of 128
partitions):

  xb   = bf16(x)                       # stats + residual
  mu_s = sum_c xb / C  (PE ones-matmul); var = E[x^2] - mu^2
  r_s  = exp(-0.5 ln(var+eps))         # ACT Ln+Exp (one act table)
  xn   = bf16((xb - mu) * r)           # fully normalized input; r folded
                                       # here so q/k/v need no later scaling
  qt/kt[o, s] = bf16(Wqk^T xn)         # transposed projections
  v[t, hd]    = bf16(xn^T Wv)          # parity-split layout (see below)
  scores[t, s] = kt-block^T-ish @ qt-block  (per head, K=64)
  P = bf16(exp(0.125*scores + gamma_t))     # gamma = 0.125 u.xn (key-side
                                            # bias, full-dim approximation)
  O^T = v-aug^T @ P accumulated over t-chunks; the augmented ones column
        yields sigma rows 64:66 of the PV psum. Even heads' data lands at
        psum rows 0-63 = final opk rows (direct ACT Copy); odd heads' data
        goes through one partition-shift DMA per (pr, sc).
  O normalized by 1/sigma (2-row broadcast matmul), per (pr, sc) so the
  pr=3 normalization tail overlaps the output projection.
  out = Wo^T opk + bocol + xb  (fp32), one DMA for the whole y.

Host-side prep: gamma/beta folded into effective weights; Wv columns
permuted to (parity, head//2, d); u = Wk'^T bq'; bocol = bo + Wo bv'.
"""

import math

import numpy as np

C = 512
S = 1024
B = 8
NH = 8
HD = 64
N_CORES = 8

_CACHE = {}


def _build_nc(repeat=1):
    import concourse.bass as bass
    import concourse.mybir as mybir
    import concourse.tile as tile
    from concourse import bacc

    f32 = mybir.dt.float32
    bf16 = mybir.dt.bfloat16
    AF = mybir.ActivationFunctionType
    OP = mybir.AluOpType

    nc = bacc.Bacc("TRN2", debug=False, num_devices=N_CORES)

    x_d = nc.declare_dram_parameter("x", [C, S], f32, isOutput=False)
    wqk_d = nc.declare_dram_parameter("wqk", [C, 2 * C], bf16, isOutput=False)
    wv_d = nc.declare_dram_parameter("wv", [C, C], bf16, isOutput=False)
    wo_d = nc.declare_dram_parameter("wo", [C, C], bf16, isOutput=False)
    ucol_d = nc.declare_dram_parameter("ucol", [128, 4], bf16, isOutput=False)
    bocol_d = nc.declare_dram_parameter("bocol", [128, 4], f32, isOutput=False)
    emat_d = nc.declare_dram_parameter("emat", [2, 128], bf16, isOutput=False)
    y_d = nc.declare_dram_parameter("y", [C, S], f32, isOutput=True)

    with tile.TileContext(nc) as tc:
        import contextlib

        with contextlib.ExitStack() as ctx:
            ctx.enter_context(nc.allow_low_precision(reason="bf16 matmul pipeline"))
            const = ctx.enter_context(tc.tile_pool(name="const", bufs=1))
            vpool = ctx.enter_context(tc.tile_pool(name="vpool", bufs=1))
            big = ctx.enter_context(
                tc.tile_pool(name="big", bufs=1 if repeat == 1 else 2)
            )
            xb_pool = ctx.enter_context(
                tc.tile_pool(name="xb", bufs=1 if repeat == 1 else 2)
            )
            xn_pool = ctx.enter_context(
                tc.tile_pool(name="xn", bufs=1 if repeat == 1 else 2)
            )
            xsq_pool = ctx.enter_context(tc.tile_pool(name="xsq", bufs=2))
            pt_pool = ctx.enter_context(
                tc.tile_pool(name="pt", bufs=3 if repeat == 1 else 2)
            )
            ost_pool = ctx.enter_context(tc.tile_pool(name="ost", bufs=2))
            yf_pool = ctx.enter_context(tc.tile_pool(name="yf", bufs=1))
            xst_pool = ctx.enter_context(tc.tile_pool(name="xst", bufs=2))
            stats_sb = ctx.enter_context(tc.tile_pool(name="stats_sb", bufs=1))
            sg_pool = ctx.enter_context(tc.tile_pool(name="sg", bufs=2))
            ps = ctx.enter_context(tc.tile_pool(name="ps", bufs=2, space="PSUM"))

            # ---- static loads ------------------------------------------------
            x_re = x_d[:, :].rearrange("(kc p) s -> p kc s", p=128)
            wqk_sb = const.tile([128, 4, 2 * C], bf16)
            wqk_re = wqk_d[:, :].rearrange("(kc p) o -> p kc o", p=128)
            nc.sync.dma_start(out=wqk_sb[:, :, 512:], in_=wqk_re[:, :, 512:])
            nc.sync.dma_start(out=wqk_sb[:, :, 0:512], in_=wqk_re[:, :, 0:512])
            wv_sb = const.tile([128, 4, C], bf16)
            nc.sync.dma_start(
                out=wv_sb[:], in_=wv_d[:, :].rearrange("(kc p) o -> p kc o", p=128)
            )
            wo_sb = const.tile([128, 4, C], bf16)
            nc.sync.dma_start(
                out=wo_sb[:], in_=wo_d[:, :].rearrange("(kc p) o -> p kc o", p=128)
            )
            ucol_sb = const.tile([128, 4], bf16)
            nc.sync.dma_start(out=ucol_sb[:], in_=ucol_d[:, :])
            bocol_sb = const.tile([128, 4], f32)
            nc.sync.dma_start(out=bocol_sb[:], in_=bocol_d[:, :])
            emat_sb = const.tile([66, 128], bf16)
            nc.sync.dma_start(out=emat_sb[64:66, :], in_=emat_d[:, :])

            ones1f = const.tile([1, 128], f32)
            nc.vector.memset(ones1f[:], 1.0)
            ones1 = const.tile([1, 128], bf16)
            nc.vector.tensor_copy(ones1[:], ones1f[:])
            onescf = const.tile([128, 1], f32)
            nc.vector.memset(onescf[:], 1.0)
            onesc = const.tile([128, 1], bf16)
            nc.vector.tensor_copy(onesc[:], onescf[:])
            epsr = const.tile([1, 1], f32)
            nc.vector.memset(epsr[:], 1e-5)

            # v layout: [128 t, tcn, parity, hh, 66]; head h = 2*hh + parity.
            # cols 0-63 data; col 64+parity holds sigma-ones (the other is 0)
            # so the PV psum rows 64:66 accumulate [sigma_even; sigma_odd].
            v_sb = vpool.tile([128, 8, 2, 4, 66], bf16)
            nc.vector.memset(v_sb[:, :, :, :, 64:66], 0.0)
            nc.vector.memset(v_sb[:, :, 0, :, 64:65], 1.0)
            nc.vector.memset(v_sb[:, :, 1, :, 65:66], 1.0)

            # initial x -> bf16, staged through small f32 tiles
            xb0 = xb_pool.tile([128, 4, S], bf16, tag="xb")
            for kc in range(4):
                for sc in range(2):
                    xstage = xst_pool.tile([128, 512], f32, tag="xst")
                    nc.sync.dma_start(
                        out=xstage[:], in_=x_re[:, kc, sc * 512 : (sc + 1) * 512]
                    )
                    nc.gpsimd.tensor_copy(
                        xb0[:, kc, sc * 512 : (sc + 1) * 512], xstage[:]
                    )

            def psA(name):
                return ps.tile([128, S], f32, tag="psA", name=name)

            def psB(name):
                return ps.tile([128, 512], f32, tag="psB", name=name)

            def psC(name):
                return ps.tile([128, 512], f32, tag="psC", name=name)

            def blk(sc):
                return slice(sc * 512, (sc + 1) * 512)

            def body(xb, dst_dram, it):
                """One attention layer: xb [128, 4, S] bf16 -> dst_dram [C, S]."""
                # ---- stats: mu, E[x^2] ---------------------------------------
                sts = [psB(f"stx{it}_{sc}") for sc in range(2)]
                for kc in range(4):
                    for sc in range(2):
                        nc.tensor.matmul(
                            sts[sc][0:1, :],
                            onesc[:],
                            xb[:, kc, blk(sc)],
                            start=(kc == 0),
                            stop=(kc == 3),
                        )
                stq = [psB(f"stq{it}_{sc}") for sc in range(2)]
                for kc in range(4):
                    for sc in range(2):
                        xsq = xsq_pool.tile([128, 512], bf16)
                        nc.gpsimd.tensor_mul(
                            xsq[:], xb[:, kc, blk(sc)], xb[:, kc, blk(sc)]
                        )
                        nc.tensor.matmul(
                            stq[sc][0:1, :],
                            onesc[:],
                            xsq[:],
                            start=(kc == 0),
                            stop=(kc == 3),
                        )
                murow_f = stats_sb.tile([1, S], f32, tag="murow_f")
                for sc in range(2):
                    nc.vector.tensor_scalar_mul(
                        murow_f[:, blk(sc)], sts[sc][0:1, :], 1.0 / C
                    )
                srowA = stats_sb.tile([1, S], f32, tag="srowA")
                for sc in range(2):
                    nc.vector.tensor_scalar_mul(
                        srowA[:, blk(sc)], stq[sc][0:1, :], 1.0 / C
                    )
                srowB = stats_sb.tile([1, S], f32, tag="srowB")
                nc.vector.tensor_mul(srowB[:], murow_f[:], murow_f[:])
                nc.vector.tensor_tensor(srowA[:], srowA[:], srowB[:], OP.subtract)
                nc.scalar.activation(srowB[:], srowA[:], AF.Ln, bias=epsr[:], scale=1.0)
                rrow = stats_sb.tile([1, S], bf16, tag="rrow")
                nc.scalar.activation(rrow[:], srowB[:], AF.Exp, bias=0.0, scale=-0.5)
                murrow = stats_sb.tile([1, S], bf16, tag="murrow")
                nc.vector.tensor_mul(murrow[:], murow_f[:], rrow[:])
                # broadcast mu*r and r to all partitions
                MuR_sb = big.tile([128, S], bf16, tag="MuR")
                R_sb = big.tile([128, S], bf16, tag="R")
                for sc in range(2):
                    pm = psB(f"mu{it}_{sc}")
                    nc.tensor.matmul(
                        pm[:], ones1[:], murrow[:, blk(sc)], start=True, stop=True
                    )
                    nc.vector.tensor_copy(MuR_sb[:, blk(sc)], pm[:])
                    pr_ = psB(f"rb{it}_{sc}")
                    nc.tensor.matmul(
                        pr_[:], ones1[:], rrow[:, blk(sc)], start=True, stop=True
                    )
                    nc.vector.tensor_copy(R_sb[:, blk(sc)], pr_[:])
                # xn = xb*r - mu*r  (fully normalized, bf16 2x-rate DVE)
                xn = xn_pool.tile([128, 4, S], bf16, tag="xn")
                for kc in range(4):
                    for sc in range(2):
                        nc.vector.tensor_tensor(
                            xn[:, kc, blk(sc)],
                            xb[:, kc, blk(sc)],
                            R_sb[:, blk(sc)],
                            OP.mult,
                        )
                        nc.vector.tensor_tensor(
                            xn[:, kc, blk(sc)],
                            xn[:, kc, blk(sc)],
                            MuR_sb[:, blk(sc)],
                            OP.subtract,
                        )
                # ---- gamma: key-side softmax bias, gcol via PE transpose ------
                gps = psA(f"g{it}")
                for sc in range(2):
                    for kc in range(4):
                        nc.tensor.matmul(
                            gps[0:1, blk(sc)],
                            ucol_sb[:, kc : kc + 1],
                            xn[:, kc, blk(sc)],
                            start=(kc == 0),
                            stop=(kc == 3),
                        )
                grow = stats_sb.tile([1, S], f32, tag="grow")
                nc.vector.tensor_scalar_mul(grow[:], gps[0:1, :], 0.125)
                tg = psB(f"tg{it}")
                for tcn in range(8):
                    nc.tensor.transpose(
                        tg[:, tcn : tcn + 1],
                        grow[0:1, tcn * 128 : (tcn + 1) * 128],
                        onescf[0:1, 0:1],
                    )
                gcol = stats_sb.tile([128, 8], f32, tag="gcol")
                nc.vector.tensor_copy(gcol[:], tg[:, 0:8])

                # ---- Q/K projections, transposed layout [o, s] ----------------
                qt_sb = big.tile([128, 4, S], bf16, tag="qt")
                kt_sb = big.tile([128, 4, S], bf16, tag="kt")

                def qk_chunk(oc, pool_fn):
                    dst = qt_sb if oc < 4 else kt_sb
                    o4 = oc % 4
                    for sc in range(2):
                        p = pool_fn(f"qk{it}_{oc}_{sc}")
                        for kc in range(4):
                            nc.tensor.matmul(
                                p[:],
                                wqk_sb[:, kc, oc * 128 : (oc + 1) * 128],
                                xn[:, kc, blk(sc)],
                                start=(kc == 0),
                                stop=(kc == 3),
                            )
                        nc.vector.tensor_copy(dst[:, o4, blk(sc)], p[:])

                for i, oc in enumerate([4, 5, 6, 7]):
                    qk_chunk(oc, psB if i % 2 == 0 else psC)

                def v_chunk(tcn):
                    p = psB(f"v{it}_{tcn}")
                    for kc in range(4):
                        nc.tensor.matmul(
                            p[:],
                            xn[:, kc, tcn * 128 : (tcn + 1) * 128],
                            wv_sb[:, kc, :],
                            start=(kc == 0),
                            stop=(kc == 3),
                        )
                    pr_ = p[:].rearrange("p (i h d) -> p i h d", i=2, h=4)
                    nc.scalar.activation(
                        v_sb[:, tcn, :, :, 0:64], pr_[:, :, :, :], AF.Copy,
                        bias=0.0, scale=1.0,
                    )

                # ---- attention ------------------------------------------------
                opk_sb = big.tile([128, 4, S], bf16, tag="opk")
                sig_st = {}

                def emit_norm(pr, sc):
                    sig_stage = sig_st[pr]
                    siginv = sg_pool.tile(
                        [66, 512], bf16, tag="siginv", name=f"siginv{it}_{pr}_{sc}"
                    )
                    nc.vector.reciprocal(siginv[64:66, :], sig_stage[64:66, blk(sc)])
                    ps_e = psB(f"pe{it}_{pr}_{sc}")
                    nc.tensor.matmul(
                        ps_e[:],
                        emat_sb[64:66, :],
                        siginv[64:66, :],
                        start=True,
                        stop=True,
                    )
                    nc.vector.tensor_tensor(
                        opk_sb[:, pr, blk(sc)], ps_e[:], opk_sb[:, pr, blk(sc)],
                        OP.mult,
                    )

                for pr in range(4):
                    qk_chunk(pr, psB if pr % 2 == 0 else psC)
                    sig_stage = sg_pool.tile(
                        [66, S], f32, tag="sigst", name=f"sigst{it}_{pr}"
                    )
                    sig_st[pr] = sig_stage
                    for sc in range(2):
                        ost = ost_pool.tile(
                            [64, 512], bf16, tag="ost", name=f"ost{it}_{pr}_{sc}"
                        )
                        pva = psC(f"pva{it}_{pr}_{sc}")
                        pvb = psC(f"pvb{it}_{pr}_{sc}")
                        for tcn in range(8):
                            if pr == 0 and sc == 0:
                                v_chunk(tcn)
                            pst = psA(f"sc{it}_{pr}_{sc}_{tcn}")
                            for hi in range(2):
                                b0 = 64 * hi
                                nc.tensor.matmul(
                                    pst[:, hi * 512 : (hi + 1) * 512],
                                    kt_sb[b0 : b0 + 64, pr, tcn * 128 : (tcn + 1) * 128],
                                    qt_sb[b0 : b0 + 64, pr, blk(sc)],
                                    start=True,
                                    stop=True,
                                )
                            pt = pt_pool.tile([128, S], bf16)
                            nc.scalar.activation(
                                pt[:],
                                pst[:],
                                AF.Exp,
                                bias=gcol[:, tcn : tcn + 1],
                                scale=0.125,
                            )
                            nc.tensor.matmul(
                                pva[0:66, :],
                                v_sb[:, tcn, 0, pr, 0:66],
                                pt[:, 0:512],
                                start=(tcn == 0),
                                stop=(tcn == 7),
                            )
                            nc.tensor.matmul(
                                pvb[0:66, :],
                                v_sb[:, tcn, 1, pr, 0:66],
                                pt[:, 512:1024],
                                start=(tcn == 0),
                                stop=(tcn == 7),
                            )
                            if pr > 0 and tcn == 4:
                                emit_norm(pr - 1, sc)
                        # evacuate PV psum: even-head data lands directly at
                        # opk rows 0-63 (ACT copy); odd-head data goes through
                        # ost + one 64-partition-shift DMA per (pr, sc).
                        nc.scalar.activation(
                            opk_sb[0:64, pr, blk(sc)], pva[0:64, :], AF.Copy,
                            bias=0.0, scale=1.0,
                        )
                        nc.scalar.activation(
                            ost[:], pvb[0:64, :], AF.Copy, bias=0.0, scale=1.0
                        )
                        nc.vector.tensor_copy(
                            sig_stage[64:66, blk(sc)], pva[64:66, :]
                        )
                        nc.vector.tensor_tensor(
                            sig_stage[64:66, blk(sc)],
                            sig_stage[64:66, blk(sc)],
                            pvb[64:66, :],
                            OP.add,
                        )
                        nc.sync.dma_start(
                            out=opk_sb[64:128, pr, blk(sc)], in_=ost[:]
                        )

                # ---- output projection + bias + residual ----------------------
                # sc-major so emit_norm(3, 1) hides under the sc=0 groups.
                nxt = None
                if it < repeat - 1:
                    nxt = xb_pool.tile([128, 4, S], bf16, tag="xb", name=f"xb{it + 1}")
                y_full = yf_pool.tile([128, 4, S], f32, tag="yfull", name=f"yf{it}")
                for sc in range(2):
                    emit_norm(3, sc)
                    for c in range(4):
                        if c % 2 == 0:
                            ps_y = psB(f"y{it}_{c}_{sc}")
                        else:
                            ps_y = psA(f"y{it}_{c}_{sc}")[:, 0:512]
                        for oc in range(4):
                            nc.tensor.matmul(
                                ps_y[:],
                                wo_sb[:, oc, c * 128 : (c + 1) * 128],
                                opk_sb[:, oc, blk(sc)],
                                start=(oc == 0),
                                stop=(oc == 3),
                            )
                        if nxt is not None:
                            nc.vector.scalar_tensor_tensor(
                                nxt[:, c, blk(sc)],
                                ps_y[:],
                                bocol_sb[:, c : c + 1],
                                xb[:, c, blk(sc)],
                                OP.add,
                                OP.add,
                            )
                        nc.vector.scalar_tensor_tensor(
                            y_full[:, c, blk(sc)],
                            ps_y[:],
                            bocol_sb[:, c : c + 1],
                            xb[:, c, blk(sc)],
                            OP.add,
                            OP.add,
                        )
                nc.sync.dma_start(
                    out=dst_dram[:, :].rearrange("(kc p) s -> p kc s", p=128),
                    in_=y_full[:, :, :],
                )
                return nxt

            cur = xb0
            for it in range(repeat):
                cur = body(cur, y_d, it)

    nc.finalize()
    return nc


def _host_prep(Wq, bq, Wk, bk, Wv, bv, Wo, bo, gamma, beta):
    import ml_dtypes

    bfd = ml_dtypes.bfloat16
    g = np.asarray(gamma, np.float64)
    be = np.asarray(beta, np.float64)

    def eff(W, b):
        W = np.asarray(W, np.float64)
        b = np.asarray(b, np.float64)
        return W * g[None, :], b + W @ be

    Wqp, bqp = eff(Wq, bq)
    Wkp, bkp = eff(Wk, bk)
    Wvp, bvp = eff(Wv, bv)

    wqk = np.concatenate([Wqp.T, Wkp.T], axis=1).astype(bfd)
    # permute Wv columns to (parity, head//2, d) order
    wvt = Wvp.T.reshape(C, 8, 64)
    perm = [2 * hh + par for par in range(2) for hh in range(4)]
    wv = np.ascontiguousarray(wvt[:, perm, :].reshape(C, C)).astype(bfd)
    u = Wkp.T @ bqp
    ucol = u.reshape(4, 128).T.astype(bfd).copy()
    wo = np.ascontiguousarray(np.asarray(Wo, np.float64).T).astype(bfd)
    bo_eff = np.asarray(bo, np.float64) + np.asarray(Wo, np.float64) @ bvp
    bocol = bo_eff.reshape(4, 128).T.astype(np.float32).copy()
    # emat rows land at stationary partitions 64 (even sigma) / 65 (odd sigma)
    emat = np.zeros((2, 128), bfd)
    emat[0, :64] = 1.0  # partition 64 = 1/sigma_even -> even head rows 0-63
    emat[1, 64:] = 1.0  # partition 65 = 1/sigma_odd -> odd head rows 64-127
    return dict(wqk=wqk, wv=wv, wo=wo, ucol=ucol, bocol=bocol, emat=emat)


def get_nc(repeat=1):
    if repeat not in _CACHE:
        _CACHE[repeat] = _build_nc(repeat)
    return _CACHE[repeat]


def make_in_maps(inputs):
    shared = _host_prep(
        inputs["Wq"], inputs["bq"], inputs["Wk"], inputs["bk"],
        inputs["Wv"], inputs["bv"], inputs["Wo"], inputs["bo"],
        inputs["gamma"], inputs["beta"],
    )
    x = np.asarray(inputs["x"], np.float32)
    in_maps = []
    for b in range(N_CORES):
        m = dict(shared)
        m["x"] = np.ascontiguousarray(x[b].reshape(C, S))
        in_maps.append(m)
    return in_maps


def kernel(**inputs):
    from concourse.bass_utils import run_bass_kernel_spmd

    nc = get_nc(repeat=1)
    in_maps = make_in_maps(inputs)
    res = run_bass_kernel_spmd(nc, in_maps, list(range(N_CORES)))
    out = np.stack([res.results[b]["y"].reshape(C, 32, 32) for b in range(N_CORES)])
    return out.astype(np.float32)
